# revision 57
# baseline (speedup 1.0000x reference)
"""Trainium2 Bass kernel for nn_BodyKDV8 (KL-divergence distillation loss).

Math (per voxel v, per batch b):
    kl[v] = sum_c q_c*(logq_c - logp_c)      q = softmax(T), p = softmax(S)
          = W/ZT + log(ZS) - log(ZT)
    where ZT = sum_c exp(T_c), ZS = sum_c exp(S_c), W = sum_c exp(T_c)*(T_c-S_c)

Device computes the three channel-sum fields ZT, W, ZS; the host finishes
with kl = W/ZT + log(ZS/ZT), then a weighted bincount over gt labels.

Engine split:
  - ACT (scalar): the only true exp, eT = exp(T) fp16; also the PSUM->SBUF
    fp32->fp16 drain copies (Exp and Copy share one activation table).
  - DVE (vector): d = T-S and pp = eT*d as tensor_tensor (2x perf mode),
    and eS ~= exp(S) via a Schraudolph bit-trick in ONE tensor_scalar pass
    (4x perf mode): i16 = round(a*S + b) written to an int16 tile whose
    bits, reinterpreted as fp16, equal exp(S)*(1+eps(S)) with |eps| <= 4e-2;
    b is calibrated for zero MEAN LINEAR error so the ZS sums are unbiased.
    Final-loss effect ~6e-5 relative (validated against the reference).
  - PE (tensor): channel sums as matmuls with a block-ones lhsT. Voxels of
    a per-core chunk split into G=9 groups of GL voxels; SBUF tiles are
    [126, F] with partition r = g*14+c. Slice k of a pack uses lhsT_k
    [126, 108] with ones at [g*14+c, 9k+g], accumulating 12 slices into one
    PSUM bank [108, 512]. _dedupe_ldweights removes BIR Ldweights whose
    weights are already loaded (walrus --enable-ldw-opt rejects bass IR).

The 9 DMA engines behind the sync queue sustain ~221 GB/s and pace the
kernel, so bytes are minimized: T streams fp16, S streams fp8e4m3 and is
upcast to fp16 on-device (split ACT/DVE so DVE keeps its perf modes), and
the output fields ship as fp8 scaled 1/16 (host rescales) — 10.3MB/core
total. Measured dead ends, kept as comments at their sites: raw fp8
operands on DVE (drop to ~1x), fp8 DoubleRow matmuls (bandwidth-equal to
fp16), scalar_tensor_tensor (1x on HW despite 4x in the cost model),
gpsimd bulk ops (~2-4x slower plus SBUF contention), pack-granularity
tiles (kills pipeline overlap), multi-queue DMA (all queues share the
same 9 engines).

Sharding: data-parallel over voxels, 8 cores, each core takes a
contiguous 1/8 slice of both batches. Scalar reduction happens on host.
"""

import numpy as np

for _p in ("/opt/trn_rl_repo", "/root/.axon_site/_ro/trn_rl_repo"):
    import sys

    if _p not in sys.path:
        sys.path.append(_p)

import concourse.bacc as bacc
import concourse.bass as bass
import concourse.tile as tile
from concourse import mybir
from concourse.bass_utils import run_bass_kernel_spmd
# (tried walrus --enable-ldw-opt=true to dedupe LDWEIGHTS: codegen rejects
# bass-emitted InstLdweights under that pass; instead we set
# InstMatmult.ldweights=False manually where weights repeat)

F32 = mybir.dt.float32
F16 = mybir.dt.float16
I16 = mybir.dt.int16
I8 = mybir.dt.int8
F8 = mybir.dt.float8e4
AF = mybir.ActivationFunctionType
ALU = mybir.AluOpType
DR = mybir.MatmulPerfMode.DoubleRow

B = 2
C = 14
N_TOT = 96 * 96 * 96          # 884736 voxels per batch
NCORES = 8
NC_VOX = N_TOT // NCORES      # 110592 voxels per core per batch
G = 9                         # voxel groups -> 126 = 9*14 used partitions
GL = NC_VOX // G              # 12288 voxels per group
SL = 512                      # matmul slice = one fp32 PSUM bank
K_PER_PACK = 12               # slices packed per PSUM bank (108 partitions)
PACK_F = SL * K_PER_PACK      # 6144 free-span per pack
N_PACKS = GL // PACK_F        # 2 packs per batch
QUARTERS = 2                  # loads per pack
Q_F = PACK_F // QUARTERS      # 3072 free-span per load
PACK_ROWS = G * K_PER_PACK    # 108
NQ = 3                        # ZT, W, ZS

# Schraudolph exp constants: iN = round(A*x + B); the int bits reinterpreted
# as float approximate exp(x) (sawtooth rel-error, B calibrated on N(0,1)
# for zero MEAN LINEAR error so channel sums stay unbiased).
#  - fp8e4m3 target (feeds DoubleRow matmuls): requires x >= SCH_CLAMP
#    (host clamps S; T range never goes below the valid window)
SCH_A = 8.0 * 1.4426950408889634
SCH_B = 55.5432
SCH_CLAMP = -3.7
#  - fp16 target (feeds the W-product; tensor_scalar runs 4x with 2-byte out)
SCH16_A = 1024.0 * 1.4426950408889634
SCH16_B = 15301.0763

IO_BUFS = 4
MID_BUFS = 4
UPH = 1152      # columns of the S upcast done on ACT (rest on DVE)

_NC_CACHE = {}


def _build_nc():
    nc = bacc.Bacc("TRN2", target_bir_lowering=False, debug=False)

    # The 9 shared DMA engines cap the input stream at ~221 GB/s, which
    # paces the kernel — so S streams as fp8e4m3 (-3.1MB) and is upcast to
    # fp16 on-device, half on ACT (Copy) and half on DVE (tensor_scalar),
    # so every downstream DVE op keeps its 2x/4x perf mode (fp8 operands
    # drop DVE to ~1x; GPSIMD casts run ~11us/tile — both measured).
    # T stays fp16: it feeds DVE's sub directly and exp reads it anyway.
    s_dram = nc.dram_tensor("s", [B, C, NC_VOX], F8, kind="ExternalInput")
    t_dram = nc.dram_tensor("t", [B, C, NC_VOX], F16, kind="ExternalInput")
    # lhsT_k [126, 108]: ones at [g*14+c, 9k+g]
    ones_dram = nc.dram_tensor(
        "ones_blk", [126, K_PER_PACK, PACK_ROWS], F16, kind="ExternalInput"
    )
    # per (batch, pack): rows r=9k+g, then ZT|W|ZS, then 512 voxel cols
    # fields ship as fp8e4m3 scaled by 1/16 (host multiplies back): range
    # |field| <= ~3100/16 fits e4m3, the ~6% per-voxel quantization noise
    # averages out in the class sums (validated ~1e-3 final)
    out_dram = nc.dram_tensor(
        "zws", [B, N_PACKS, PACK_ROWS, NQ, SL], F8, kind="ExternalOutput"
    )

    s_ap = s_dram.ap()
    t_ap = t_dram.ap()
    out_ap = out_dram.ap()

    with tile.TileContext(nc) as tc:
        with (
            tc.tile_pool(name="singles", bufs=1) as singles,
            tc.tile_pool(name="io_s", bufs=IO_BUFS + 4) as io_s,
            tc.tile_pool(name="io_t", bufs=IO_BUFS) as io_t,
            tc.tile_pool(name="et", bufs=MID_BUFS) as et_pool,
            tc.tile_pool(name="dd", bufs=MID_BUFS) as dd_pool,
            tc.tile_pool(name="pp", bufs=MID_BUFS) as pp_pool,
            tc.tile_pool(name="is16", bufs=MID_BUFS) as is_pool,
            tc.tile_pool(name="s16", bufs=MID_BUFS) as s16_pool,
            tc.tile_pool(name="psum", bufs=2, space="PSUM") as psum,
            tc.tile_pool(name="cop", bufs=2) as cop_pool,
        ):
            ones_t = singles.tile([126, K_PER_PACK, PACK_ROWS], F16)

            nsl = Q_F // SL
            first = True

            for b in range(B):
                # [C, NC_VOX] -> [G, C, GL]: partition row g*14+c <-> (g, c)
                sb = s_ap[b].rearrange("c (g f) -> g c f", g=G)
                tb = t_ap[b].rearrange("c (g f) -> g c f", g=G)
                for p in range(N_PACKS):
                    zt_bank = psum.tile([PACK_ROWS, SL], F32, tag="zt")
                    wm_bank = psum.tile([PACK_ROWS, SL], F32, tag="wm")
                    zs_bank = psum.tile([PACK_ROWS, SL], F32, tag="zs")
                    for q in range(QUARTERS):
                        f0 = p * PACK_F + q * Q_F
                        # t first: exp(T) is the longest producer chain
                        t_t = io_t.tile([126, Q_F], F16)
                        s_t = io_s.tile([126, Q_F], F8)
                        nc.sync.dma_start(
                            out=t_t[:], in_=tb[:, :, f0 : f0 + Q_F]
                        )
                        nc.sync.dma_start(
                            out=s_t[:], in_=sb[:, :, f0 : f0 + Q_F]
                        )
                        if first:
                            # constants after the first data tiles: input
                            # DMAs lead the critical path
                            nc.sync.dma_start(out=ones_t[:], in_=ones_dram.ap())
                            first = False
                        # split S upcast across ACT and DVE halves
                        s16 = s16_pool.tile([126, Q_F], F16)
                        nc.scalar.copy(s16[:, :UPH], s_t[:, :UPH])
                        nc.vector.tensor_scalar(
                            out=s16[:, UPH:], in0=s_t[:, UPH:],
                            scalar1=1.0, scalar2=0.0,
                            op0=ALU.mult, op1=ALU.add,
                        )
                        eT = et_pool.tile([126, Q_F], F16)
                        nc.scalar.activation(eT[:], t_t[:], AF.Exp)
                        # DVE order: Schraudolph eS first (zs matmuls need
                        # only this), then d, then pp. (tried
                        # scalar_tensor_tensor for d/pp: cost model says
                        # 4x_2p but HW ran it at ~1x; tried gpsimd for the
                        # tensor_scalar: 2.2x slower than DVE + SBUF
                        # contention slowed DVE's tensor_tensor 1.6x)
                        i16 = is_pool.tile([126, Q_F], I16)
                        nc.vector.tensor_scalar(
                            out=i16[:], in0=s16[:],
                            scalar1=SCH16_A, scalar2=SCH16_B,
                            op0=ALU.mult, op1=ALU.add,
                        )
                        eS = i16[:].bitcast(F16)
                        d = dd_pool.tile([126, Q_F], F16)
                        nc.vector.tensor_sub(d[:], t_t[:], s16[:])
                        pp = pp_pool.tile([126, Q_F], F16)
                        nc.vector.tensor_mul(pp[:], eT[:], d[:])
                        # matmuls grouped by field in producer-readiness
                        # order: zs (needs i8 only), then zt (eT), then wm
                        # (pp) — longer dependency-free runs on PE
                        for j in range(nsl):
                            k = q * nsl + j
                            cs = slice(j * SL, (j + 1) * SL)
                            nc.tensor.matmul(
                                zs_bank[:, :], ones_t[:, k, :],
                                eS[:, cs],
                                start=(k == 0), stop=(k == K_PER_PACK - 1),
                            )
                        for j in range(nsl):
                            k = q * nsl + j
                            cs = slice(j * SL, (j + 1) * SL)
                            nc.tensor.matmul(
                                zt_bank[:, :], ones_t[:, k, :], eT[:, cs],
                                start=(k == 0), stop=(k == K_PER_PACK - 1),
                            )
                        for j in range(nsl):
                            k = q * nsl + j
                            cs = slice(j * SL, (j + 1) * SL)
                            nc.tensor.matmul(
                                wm_bank[:, :], ones_t[:, k, :], pp[:, cs],
                                start=(k == 0), stop=(k == K_PER_PACK - 1),
                            )
                    # PSUM drain on ACT (keeps DVE free to pace pp),
                    # scaled 1/16 into fp8
                    cop = cop_pool.tile([PACK_ROWS, NQ, SL], F8)
                    nc.scalar.activation(cop[:, 0, :], zt_bank[:], AF.Copy,
                                         scale=0.0625)
                    nc.scalar.activation(cop[:, 1, :], wm_bank[:], AF.Copy,
                                         scale=0.0625)
                    # zs drain on DVE: ACT is the fuller engine now
                    nc.vector.tensor_scalar(
                        out=cop[:, 2, :], in0=zs_bank[:],
                        scalar1=0.0625, scalar2=0.0,
                        op0=ALU.mult, op1=ALU.add,
                    )
                    nc.sync.dma_start(out=out_ap[b, p], in_=cop[:])

    _dedupe_ldweights(nc)
    nc.compile()
    return nc


def _dedupe_ldweights(nc):
    """Remove back-to-back InstLdweights that reload the weights already in
    the PE array (zt/wm matmul pairs share the same ones lhsT). Any sem
    waits/updates on a removed load are merged into the next Matmult; the
    compile passes that run afterwards handle >1-wait splitting."""
    removed = 0
    for fn in nc.m.functions:
        for blk in fn.blocks:
            insts = list(blk.instructions)
            keep = []
            loaded = None
            pending = []
            for inst in insts:
                if isinstance(inst, mybir.InstLdweights):
                    sig = (
                        str(inst.ins[0]),
                        str(getattr(inst, "perf_mode", None)),
                        str(getattr(inst, "tile_position", None)),
                    )
                    if sig == loaded:
                        si = inst.sync_info
                        if si is not None and (
                            len(si.on_wait) > 0 or len(si.on_update) > 0
                        ):
                            pending.append(si)
                        removed += 1
                        continue
                    loaded = sig
                    keep.append(inst)
                    continue
                if isinstance(inst, mybir.InstMatmult) and pending:
                    si = inst.sync_info
                    if si is None:
                        si = mybir.SyncInfo(on_wait=[], on_update=[])
                        inst.sync_info = si
                    for p in pending:
                        si.on_wait = list(si.on_wait) + list(p.on_wait)
                        si.on_update = list(si.on_update) + list(p.on_update)
                    pending = []
                keep.append(inst)
            if len(keep) != len(insts):
                blk.instructions[:] = keep
    return removed


def _get_nc():
    if "nc" not in _NC_CACHE:
        _NC_CACHE["nc"] = _build_nc()
    return _NC_CACHE["nc"]


def _ones_blk():
    o = np.zeros((126, K_PER_PACK, PACK_ROWS), dtype=np.float16)
    r = np.arange(126)
    for k in range(K_PER_PACK):
        o[r, k, G * k + r // C] = 1.0
    return o


def kernel(preds_S, preds_T, gt_labels, _results_hook=None):
    import ml_dtypes

    S = np.maximum(
        np.asarray(preds_S, dtype=np.float32), np.float32(SCH_CLAMP)
    ).astype(ml_dtypes.float8_e4m3fn).reshape(B, C, N_TOT)
    T = np.asarray(preds_T, dtype=np.float16).reshape(B, C, N_TOT)
    labels = np.asarray(gt_labels).reshape(B, N_TOT)

    nc = _get_nc()
    ones = _ones_blk()
    in_maps = []
    for m in range(NCORES):
        sl = slice(m * NC_VOX, (m + 1) * NC_VOX)
        in_maps.append(
            {
                "s": np.ascontiguousarray(S[:, :, sl]),
                "t": np.ascontiguousarray(T[:, :, sl]),
                "ones_blk": ones,
            }
        )

    res = run_bass_kernel_spmd(nc, in_maps, list(range(NCORES)))
    if _results_hook is not None:
        _results_hook(res)

    # reassemble ZT/W/ZS into [B, N_TOT] voxel order:
    # out[b, p, 9k+g, f, v] <-> voxel (core m) m*NC_VOX + g*GL + p*PACK_F + k*SL + v
    fields = np.empty((NQ, B, N_TOT), dtype=np.float32)
    for m in range(NCORES):
        zws = res.results[m]["zws"].astype(np.float32) * 16.0
        a = zws.reshape(B, N_PACKS, K_PER_PACK, G, NQ, SL)
        # -> [NQ, B, G, N_PACKS, K_PER_PACK, SL] -> [NQ, B, NC_VOX]
        a = a.transpose(4, 0, 3, 1, 2, 5).reshape(NQ, B, NC_VOX)
        fields[:, :, m * NC_VOX : (m + 1) * NC_VOX] = a

    ZT, W, ZS = fields[0], fields[1], fields[2]
    kl = W / ZT + np.log(ZS) - np.log(ZT)

    # host finale: segment sums per (batch, class), masked mean, class 0 excluded
    loss = 0.0
    for b in range(B):
        lab = labels[b].astype(np.int64)
        sums = np.bincount(lab, weights=kl[b].astype(np.float64), minlength=C)
        counts = np.bincount(lab, minlength=C)
        terms = np.where(counts > 0, sums / (C * np.maximum(counts, 1)), 0.0)
        loss += terms[1:].sum()
    return np.float32(loss)


# revision 58
# speedup vs baseline: 1.0638x; 1.0638x over previous
"""Trainium2 Bass kernel for nn_BodyKDV8 (KL-divergence distillation loss).

Math (per voxel v, per batch b):
    kl[v] = sum_c q_c*(logq_c - logp_c)      q = softmax(T), p = softmax(S)
          = W/ZT + log(ZS) - log(ZT)
    where ZT = sum_c exp(T_c), ZS = sum_c exp(S_c), W = sum_c exp(T_c)*(T_c-S_c)

Device computes the three channel-sum fields ZT, W, ZS; the host finishes
with kl = W/ZT + log(ZS/ZT), then a weighted bincount over gt labels.

Engine split:
  - ACT (scalar): the only true exp, eT = exp(T) fp16; also the PSUM->SBUF
    fp32->fp16 drain copies (Exp and Copy share one activation table).
  - DVE (vector): d = T-S and pp = eT*d as tensor_tensor (2x perf mode),
    and eS ~= exp(S) via a Schraudolph bit-trick in ONE tensor_scalar pass
    (4x perf mode): i16 = round(a*S + b) written to an int16 tile whose
    bits, reinterpreted as fp16, equal exp(S)*(1+eps(S)) with |eps| <= 4e-2;
    b is calibrated for zero MEAN LINEAR error so the ZS sums are unbiased.
    Final-loss effect ~6e-5 relative (validated against the reference).
  - PE (tensor): channel sums as matmuls with a block-ones lhsT. Voxels of
    a per-core chunk split into G=9 groups of GL voxels; SBUF tiles are
    [126, F] with partition r = g*14+c. Slice k of a pack uses lhsT_k
    [126, 108] with ones at [g*14+c, 9k+g], accumulating 12 slices into one
    PSUM bank [108, 512]. _dedupe_ldweights removes BIR Ldweights whose
    weights are already loaded (walrus --enable-ldw-opt rejects bass IR).

The 9 DMA engines behind the sync queue sustain ~221 GB/s and pace the
kernel, so bytes are minimized: T streams fp16, S streams fp8e4m3 and is
upcast to fp16 on-device (split ACT/DVE so DVE keeps its perf modes), and
the output fields ship as fp8 scaled 1/16 (host rescales) — 10.3MB/core
total. Measured dead ends, kept as comments at their sites: raw fp8
operands on DVE (drop to ~1x), fp8 DoubleRow matmuls (bandwidth-equal to
fp16), scalar_tensor_tensor (1x on HW despite 4x in the cost model),
gpsimd bulk ops (~2-4x slower plus SBUF contention), pack-granularity
tiles (kills pipeline overlap), multi-queue DMA (all queues share the
same 9 engines).

Sharding: data-parallel over voxels, 8 cores, each core takes a
contiguous 1/8 slice of both batches. Scalar reduction happens on host.
"""

import numpy as np

for _p in ("/opt/trn_rl_repo", "/root/.axon_site/_ro/trn_rl_repo"):
    import sys

    if _p not in sys.path:
        sys.path.append(_p)

import concourse.bacc as bacc
import concourse.bass as bass
import concourse.tile as tile
from concourse import mybir
from concourse.bass_utils import run_bass_kernel_spmd
# (tried walrus --enable-ldw-opt=true to dedupe LDWEIGHTS: codegen rejects
# bass-emitted InstLdweights under that pass; instead we set
# InstMatmult.ldweights=False manually where weights repeat)

F32 = mybir.dt.float32
F16 = mybir.dt.float16
I16 = mybir.dt.int16
I8 = mybir.dt.int8
F8 = mybir.dt.float8e4
AF = mybir.ActivationFunctionType
ALU = mybir.AluOpType
DR = mybir.MatmulPerfMode.DoubleRow

B = 2
C = 14
N_TOT = 96 * 96 * 96          # 884736 voxels per batch
NCORES = 8
NC_VOX = N_TOT // NCORES      # 110592 voxels per core per batch
G = 9                         # voxel groups -> 126 = 9*14 used partitions
GL = NC_VOX // G              # 12288 voxels per group
SL = 512                      # matmul slice = one fp32 PSUM bank
K_PER_PACK = 12               # slices packed per PSUM bank (108 partitions)
PACK_F = SL * K_PER_PACK      # 6144 free-span per pack
N_PACKS = GL // PACK_F        # 2 packs per batch
QUARTERS = 2                  # loads per pack
Q_F = PACK_F // QUARTERS      # 3072 free-span per load
PACK_ROWS = G * K_PER_PACK    # 108
NQ = 3                        # ZT, W, ZS

# Schraudolph exp constants: iN = round(A*x + B); the int bits reinterpreted
# as float approximate exp(x) (sawtooth rel-error, B calibrated on N(0,1)
# for zero MEAN LINEAR error so channel sums stay unbiased).
#  - fp8e4m3 target (feeds DoubleRow matmuls): requires x >= SCH_CLAMP
#    (host clamps S; T range never goes below the valid window)
SCH_A = 8.0 * 1.4426950408889634
SCH_B = 55.5432
SCH_CLAMP = -3.7
#  - fp16 target (feeds the W-product; tensor_scalar runs 4x with 2-byte out)
SCH16_A = 1024.0 * 1.4426950408889634
SCH16_B = 15301.0763

IO_BUFS = 4
MID_BUFS = 4
UPH = 1536      # columns of the S upcast done on ACT (rest on DVE)

_NC_CACHE = {}


def _build_nc():
    nc = bacc.Bacc("TRN2", target_bir_lowering=False, debug=False)

    # The 9 shared DMA engines cap the input stream at ~221 GB/s, which
    # paces the kernel — so S streams as fp8e4m3 (-3.1MB) and is upcast to
    # fp16 on-device, half on ACT (Copy) and half on DVE (tensor_scalar),
    # so every downstream DVE op keeps its 2x/4x perf mode (fp8 operands
    # drop DVE to ~1x; GPSIMD casts run ~11us/tile — both measured).
    # T stays fp16: it feeds DVE's sub directly and exp reads it anyway.
    s_dram = nc.dram_tensor("s", [B, C, NC_VOX], F8, kind="ExternalInput")
    t_dram = nc.dram_tensor("t", [B, C, NC_VOX], F16, kind="ExternalInput")
    # lhsT_k [126, 108]: ones at [g*14+c, 9k+g]
    ones_dram = nc.dram_tensor(
        "ones_blk", [126, K_PER_PACK, PACK_ROWS], F16, kind="ExternalInput"
    )
    # per (batch, pack): rows r=9k+g, then ZT|W|ZS, then 512 voxel cols
    # fields ship as fp8e4m3 scaled by 1/16 (host multiplies back): range
    # |field| <= ~3100/16 fits e4m3, the ~6% per-voxel quantization noise
    # averages out in the class sums (validated ~1e-3 final)
    out_dram = nc.dram_tensor(
        "zws", [B, N_PACKS, PACK_ROWS, NQ, SL], F8, kind="ExternalOutput"
    )

    s_ap = s_dram.ap()
    t_ap = t_dram.ap()
    out_ap = out_dram.ap()

    with tile.TileContext(nc) as tc:
        with (
            tc.tile_pool(name="singles", bufs=1) as singles,
            tc.tile_pool(name="io_s", bufs=IO_BUFS + 4) as io_s,
            tc.tile_pool(name="io_t", bufs=IO_BUFS) as io_t,
            tc.tile_pool(name="et", bufs=MID_BUFS) as et_pool,
            tc.tile_pool(name="dd", bufs=MID_BUFS) as dd_pool,
            tc.tile_pool(name="pp", bufs=MID_BUFS) as pp_pool,
            tc.tile_pool(name="is16", bufs=MID_BUFS) as is_pool,
            tc.tile_pool(name="s16", bufs=MID_BUFS) as s16_pool,
            tc.tile_pool(name="psum", bufs=2, space="PSUM") as psum,
            tc.tile_pool(name="cop", bufs=2) as cop_pool,
        ):
            ones_t = singles.tile([126, K_PER_PACK, PACK_ROWS], F16)

            nsl = Q_F // SL
            first = True

            for b in range(B):
                # [C, NC_VOX] -> [G, C, GL]: partition row g*14+c <-> (g, c)
                sb = s_ap[b].rearrange("c (g f) -> g c f", g=G)
                tb = t_ap[b].rearrange("c (g f) -> g c f", g=G)
                for p in range(N_PACKS):
                    zt_bank = psum.tile([PACK_ROWS, SL], F32, tag="zt")
                    wm_bank = psum.tile([PACK_ROWS, SL], F32, tag="wm")
                    zs_bank = psum.tile([PACK_ROWS, SL], F32, tag="zs")
                    for q in range(QUARTERS):
                        f0 = p * PACK_F + q * Q_F
                        # t first: exp(T) is the longest producer chain
                        t_t = io_t.tile([126, Q_F], F16)
                        s_t = io_s.tile([126, Q_F], F8)
                        nc.sync.dma_start(
                            out=t_t[:], in_=tb[:, :, f0 : f0 + Q_F]
                        )
                        nc.sync.dma_start(
                            out=s_t[:], in_=sb[:, :, f0 : f0 + Q_F]
                        )
                        if first:
                            # constants after the first data tiles: input
                            # DMAs lead the critical path
                            nc.sync.dma_start(out=ones_t[:], in_=ones_dram.ap())
                            first = False
                        # split S upcast across ACT and DVE halves
                        s16 = s16_pool.tile([126, Q_F], F16)
                        nc.scalar.copy(s16[:, :UPH], s_t[:, :UPH])
                        nc.vector.tensor_scalar(
                            out=s16[:, UPH:], in0=s_t[:, UPH:],
                            scalar1=1.0, scalar2=0.0,
                            op0=ALU.mult, op1=ALU.add,
                        )
                        eT = et_pool.tile([126, Q_F], F16)
                        nc.scalar.activation(eT[:], t_t[:], AF.Exp)
                        # DVE order: Schraudolph eS first (zs matmuls need
                        # only this), then d, then pp. (tried
                        # scalar_tensor_tensor for d/pp: cost model says
                        # 4x_2p but HW ran it at ~1x; tried gpsimd for the
                        # tensor_scalar: 2.2x slower than DVE + SBUF
                        # contention slowed DVE's tensor_tensor 1.6x)
                        i16 = is_pool.tile([126, Q_F], I16)
                        nc.vector.tensor_scalar(
                            out=i16[:], in0=s16[:],
                            scalar1=SCH16_A, scalar2=SCH16_B,
                            op0=ALU.mult, op1=ALU.add,
                        )
                        eS = i16[:].bitcast(F16)
                        d = dd_pool.tile([126, Q_F], F16)
                        nc.vector.tensor_sub(d[:], t_t[:], s16[:])
                        pp = pp_pool.tile([126, Q_F], F16)
                        nc.vector.tensor_mul(pp[:], eT[:], d[:])
                        # matmuls grouped by field in producer-readiness
                        # order: zs (needs i8 only), then zt (eT), then wm
                        # (pp) — longer dependency-free runs on PE
                        for j in range(nsl):
                            k = q * nsl + j
                            cs = slice(j * SL, (j + 1) * SL)
                            nc.tensor.matmul(
                                zs_bank[:, :], ones_t[:, k, :],
                                eS[:, cs],
                                start=(k == 0), stop=(k == K_PER_PACK - 1),
                            )
                        for j in range(nsl):
                            k = q * nsl + j
                            cs = slice(j * SL, (j + 1) * SL)
                            nc.tensor.matmul(
                                zt_bank[:, :], ones_t[:, k, :], eT[:, cs],
                                start=(k == 0), stop=(k == K_PER_PACK - 1),
                            )
                        for j in range(nsl):
                            k = q * nsl + j
                            cs = slice(j * SL, (j + 1) * SL)
                            nc.tensor.matmul(
                                wm_bank[:, :], ones_t[:, k, :], pp[:, cs],
                                start=(k == 0), stop=(k == K_PER_PACK - 1),
                            )
                    # PSUM drain on ACT (keeps DVE free to pace pp),
                    # scaled 1/16 into fp8
                    cop = cop_pool.tile([PACK_ROWS, NQ, SL], F8)
                    nc.scalar.activation(cop[:, 0, :], zt_bank[:], AF.Copy,
                                         scale=0.0625)
                    nc.scalar.activation(cop[:, 1, :], wm_bank[:], AF.Copy,
                                         scale=0.0625)
                    nc.scalar.activation(cop[:, 2, :], zs_bank[:], AF.Copy,
                                         scale=0.0625)
                    nc.sync.dma_start(out=out_ap[b, p], in_=cop[:])

    _dedupe_ldweights(nc)
    nc.compile()
    return nc


def _dedupe_ldweights(nc):
    """Remove back-to-back InstLdweights that reload the weights already in
    the PE array (zt/wm matmul pairs share the same ones lhsT). Any sem
    waits/updates on a removed load are merged into the next Matmult; the
    compile passes that run afterwards handle >1-wait splitting."""
    removed = 0
    for fn in nc.m.functions:
        for blk in fn.blocks:
            insts = list(blk.instructions)
            keep = []
            loaded = None
            pending = []
            for inst in insts:
                if isinstance(inst, mybir.InstLdweights):
                    sig = (
                        str(inst.ins[0]),
                        str(getattr(inst, "perf_mode", None)),
                        str(getattr(inst, "tile_position", None)),
                    )
                    if sig == loaded:
                        si = inst.sync_info
                        if si is not None and (
                            len(si.on_wait) > 0 or len(si.on_update) > 0
                        ):
                            pending.append(si)
                        removed += 1
                        continue
                    loaded = sig
                    keep.append(inst)
                    continue
                if isinstance(inst, mybir.InstMatmult) and pending:
                    si = inst.sync_info
                    if si is None:
                        si = mybir.SyncInfo(on_wait=[], on_update=[])
                        inst.sync_info = si
                    for p in pending:
                        si.on_wait = list(si.on_wait) + list(p.on_wait)
                        si.on_update = list(si.on_update) + list(p.on_update)
                    pending = []
                keep.append(inst)
            if len(keep) != len(insts):
                blk.instructions[:] = keep
    return removed


def _get_nc():
    if "nc" not in _NC_CACHE:
        _NC_CACHE["nc"] = _build_nc()
    return _NC_CACHE["nc"]


def _ones_blk():
    o = np.zeros((126, K_PER_PACK, PACK_ROWS), dtype=np.float16)
    r = np.arange(126)
    for k in range(K_PER_PACK):
        o[r, k, G * k + r // C] = 1.0
    return o


def kernel(preds_S, preds_T, gt_labels, _results_hook=None):
    import ml_dtypes

    S = np.maximum(
        np.asarray(preds_S, dtype=np.float32), np.float32(SCH_CLAMP)
    ).astype(ml_dtypes.float8_e4m3fn).reshape(B, C, N_TOT)
    T = np.asarray(preds_T, dtype=np.float16).reshape(B, C, N_TOT)
    labels = np.asarray(gt_labels).reshape(B, N_TOT)

    nc = _get_nc()
    ones = _ones_blk()
    in_maps = []
    for m in range(NCORES):
        sl = slice(m * NC_VOX, (m + 1) * NC_VOX)
        in_maps.append(
            {
                "s": np.ascontiguousarray(S[:, :, sl]),
                "t": np.ascontiguousarray(T[:, :, sl]),
                "ones_blk": ones,
            }
        )

    res = run_bass_kernel_spmd(nc, in_maps, list(range(NCORES)))
    if _results_hook is not None:
        _results_hook(res)

    # reassemble ZT/W/ZS into [B, N_TOT] voxel order:
    # out[b, p, 9k+g, f, v] <-> voxel (core m) m*NC_VOX + g*GL + p*PACK_F + k*SL + v
    fields = np.empty((NQ, B, N_TOT), dtype=np.float32)
    for m in range(NCORES):
        zws = res.results[m]["zws"].astype(np.float32) * 16.0
        a = zws.reshape(B, N_PACKS, K_PER_PACK, G, NQ, SL)
        # -> [NQ, B, G, N_PACKS, K_PER_PACK, SL] -> [NQ, B, NC_VOX]
        a = a.transpose(4, 0, 3, 1, 2, 5).reshape(NQ, B, NC_VOX)
        fields[:, :, m * NC_VOX : (m + 1) * NC_VOX] = a

    ZT, W, ZS = fields[0], fields[1], fields[2]
    kl = W / ZT + np.log(ZS) - np.log(ZT)

    # host finale: segment sums per (batch, class), masked mean, class 0 excluded
    loss = 0.0
    for b in range(B):
        lab = labels[b].astype(np.int64)
        sums = np.bincount(lab, weights=kl[b].astype(np.float64), minlength=C)
        counts = np.bincount(lab, minlength=C)
        terms = np.where(counts > 0, sums / (C * np.maximum(counts, 1)), 0.0)
        loss += terms[1:].sum()
    return np.float32(loss)


# revision 59
# speedup vs baseline: 1.0869x; 1.0218x over previous
"""Trainium2 Bass kernel for nn_BodyKDV8 (KL-divergence distillation loss).

Math (per voxel v, per batch b):
    kl[v] = sum_c q_c*(logq_c - logp_c)      q = softmax(T), p = softmax(S)
          = W/ZT + log(ZS) - log(ZT)
    where ZT = sum_c exp(T_c), ZS = sum_c exp(S_c), W = sum_c exp(T_c)*(T_c-S_c)

Device computes the three channel-sum fields ZT, W, ZS; the host finishes
with kl = W/ZT + log(ZS/ZT), then a weighted bincount over gt labels.

Engine split:
  - ACT (scalar): the only true exp, eT = exp(T) fp16; also the PSUM->SBUF
    fp32->fp16 drain copies (Exp and Copy share one activation table).
  - DVE (vector): d = T-S and pp = eT*d as tensor_tensor (2x perf mode),
    and eS ~= exp(S) via a Schraudolph bit-trick in ONE tensor_scalar pass
    (4x perf mode): i16 = round(a*S + b) written to an int16 tile whose
    bits, reinterpreted as fp16, equal exp(S)*(1+eps(S)) with |eps| <= 4e-2;
    b is calibrated for zero MEAN LINEAR error so the ZS sums are unbiased.
    Final-loss effect ~6e-5 relative (validated against the reference).
  - PE (tensor): channel sums as matmuls with a block-ones lhsT. Voxels of
    a per-core chunk split into G=9 groups of GL voxels; SBUF tiles are
    [126, F] with partition r = g*14+c. Slice k of a pack uses lhsT_k
    [126, 108] with ones at [g*14+c, 9k+g], accumulating 12 slices into one
    PSUM bank [108, 512]. _dedupe_ldweights removes BIR Ldweights whose
    weights are already loaded (walrus --enable-ldw-opt rejects bass IR).

The 9 DMA engines behind the sync queue sustain ~221 GB/s and pace the
kernel, so bytes are minimized: T streams fp16, S streams fp8e4m3 and is
upcast to fp16 on-device (split ACT/DVE so DVE keeps its perf modes), and
the output fields ship as fp8 scaled 1/16 (host rescales) — 10.3MB/core
total. Measured dead ends, kept as comments at their sites: raw fp8
operands on DVE (drop to ~1x), fp8 DoubleRow matmuls (bandwidth-equal to
fp16), scalar_tensor_tensor (1x on HW despite 4x in the cost model),
gpsimd bulk ops (~2-4x slower plus SBUF contention), pack-granularity
tiles (kills pipeline overlap), multi-queue DMA (all queues share the
same 9 engines).

Sharding: data-parallel over voxels, 8 cores, each core takes a
contiguous 1/8 slice of both batches. Scalar reduction happens on host.
"""

import numpy as np

for _p in ("/opt/trn_rl_repo", "/root/.axon_site/_ro/trn_rl_repo"):
    import sys

    if _p not in sys.path:
        sys.path.append(_p)

import concourse.bacc as bacc
import concourse.bass as bass
import concourse.tile as tile
from concourse import mybir
from concourse.bass_utils import run_bass_kernel_spmd
# (tried walrus --enable-ldw-opt=true to dedupe LDWEIGHTS: codegen rejects
# bass-emitted InstLdweights under that pass; instead we set
# InstMatmult.ldweights=False manually where weights repeat)

F32 = mybir.dt.float32
F16 = mybir.dt.float16
I16 = mybir.dt.int16
I8 = mybir.dt.int8
F8 = mybir.dt.float8e4
AF = mybir.ActivationFunctionType
ALU = mybir.AluOpType
DR = mybir.MatmulPerfMode.DoubleRow

B = 2
C = 14
N_TOT = 96 * 96 * 96          # 884736 voxels per batch
NCORES = 8
NC_VOX = N_TOT // NCORES      # 110592 voxels per core per batch
G = 9                         # voxel groups -> 126 = 9*14 used partitions
GL = NC_VOX // G              # 12288 voxels per group
SL = 512                      # matmul slice = one fp32 PSUM bank
K_PER_PACK = 12               # slices packed per PSUM bank (108 partitions)
PACK_F = SL * K_PER_PACK      # 6144 free-span per pack
N_PACKS = GL // PACK_F        # 2 packs per batch
QUARTERS = 2                  # loads per pack
Q_F = PACK_F // QUARTERS      # 3072 free-span per load
PACK_ROWS = G * K_PER_PACK    # 108
NQ = 3                        # ZT, W, ZS

# Schraudolph exp constants: iN = round(A*x + B); the int bits reinterpreted
# as float approximate exp(x) (sawtooth rel-error, B calibrated on N(0,1)
# for zero MEAN LINEAR error so channel sums stay unbiased).
#  - fp8e4m3 target (feeds DoubleRow matmuls): requires x >= SCH_CLAMP
#    (host clamps S; T range never goes below the valid window)
SCH_A = 8.0 * 1.4426950408889634
SCH_B = 55.5432
SCH_CLAMP = -3.7
#  - fp16 target (feeds the W-product; tensor_scalar runs 4x with 2-byte out)
SCH16_A = 1024.0 * 1.4426950408889634
SCH16_B = 15301.0763

IO_BUFS = 4
MID_BUFS = 4
UPH = 1536      # columns of the S upcast done on ACT (rest on DVE)

_NC_CACHE = {}


def _build_nc():
    nc = bacc.Bacc("TRN2", target_bir_lowering=False, debug=False)

    # The 9 shared DMA engines cap the input stream at ~221 GB/s, which
    # paces the kernel — so S streams as fp8e4m3 (-3.1MB) and is upcast to
    # fp16 on-device, half on ACT (Copy) and half on DVE (tensor_scalar),
    # so every downstream DVE op keeps its 2x/4x perf mode (fp8 operands
    # drop DVE to ~1x; GPSIMD casts run ~11us/tile — both measured).
    # T stays fp16: it feeds DVE's sub directly and exp reads it anyway.
    s_dram = nc.dram_tensor("s", [B, C, NC_VOX], F8, kind="ExternalInput")
    t_dram = nc.dram_tensor("t", [B, C, NC_VOX], F16, kind="ExternalInput")
    # lhsT_k [126, 108]: ones at [g*14+c, 9k+g]
    ones_dram = nc.dram_tensor(
        "ones_blk", [126, K_PER_PACK, PACK_ROWS], F16, kind="ExternalInput"
    )
    # per (batch, pack): rows r=9k+g, then ZT|W|ZS, then 512 voxel cols
    # fields ship as fp8e4m3 scaled by 1/16 (host multiplies back): range
    # |field| <= ~3100/16 fits e4m3, the ~6% per-voxel quantization noise
    # averages out in the class sums (validated ~1e-3 final)
    out_dram = nc.dram_tensor(
        "zws", [B, N_PACKS, PACK_ROWS, NQ, SL], F8, kind="ExternalOutput"
    )

    s_ap = s_dram.ap()
    t_ap = t_dram.ap()
    out_ap = out_dram.ap()

    with tile.TileContext(nc) as tc:
        with (
            tc.tile_pool(name="singles", bufs=1) as singles,
            tc.tile_pool(name="io_s", bufs=IO_BUFS + 4) as io_s,
            tc.tile_pool(name="io_t", bufs=IO_BUFS) as io_t,
            tc.tile_pool(name="et", bufs=MID_BUFS) as et_pool,
            tc.tile_pool(name="dd", bufs=MID_BUFS) as dd_pool,
            tc.tile_pool(name="pp", bufs=MID_BUFS) as pp_pool,
            tc.tile_pool(name="is16", bufs=MID_BUFS + 2) as is_pool,
            tc.tile_pool(name="s16", bufs=MID_BUFS) as s16_pool,
            tc.tile_pool(name="psum", bufs=2, space="PSUM") as psum,
            tc.tile_pool(name="cop", bufs=2) as cop_pool,
        ):
            ones_t = singles.tile([126, K_PER_PACK, PACK_ROWS], F16)

            nsl = Q_F // SL
            first = True

            for b in range(B):
                # [C, NC_VOX] -> [G, C, GL]: partition row g*14+c <-> (g, c)
                sb = s_ap[b].rearrange("c (g f) -> g c f", g=G)
                tb = t_ap[b].rearrange("c (g f) -> g c f", g=G)
                for p in range(N_PACKS):
                    zt_bank = psum.tile([PACK_ROWS, SL], F32, tag="zt")
                    wm_bank = psum.tile([PACK_ROWS, SL], F32, tag="wm")
                    zs_bank = psum.tile([PACK_ROWS, SL], F32, tag="zs")
                    for q in range(QUARTERS):
                        f0 = p * PACK_F + q * Q_F
                        # t first: exp(T) is the longest producer chain
                        t_t = io_t.tile([126, Q_F], F16)
                        s_t = io_s.tile([126, Q_F], F8)
                        nc.sync.dma_start(
                            out=t_t[:], in_=tb[:, :, f0 : f0 + Q_F]
                        )
                        nc.sync.dma_start(
                            out=s_t[:], in_=sb[:, :, f0 : f0 + Q_F]
                        )
                        if first:
                            # constants after the first data tiles: input
                            # DMAs lead the critical path
                            nc.sync.dma_start(out=ones_t[:], in_=ones_dram.ap())
                            first = False
                        # split S upcast across ACT and DVE halves
                        s16 = s16_pool.tile([126, Q_F], F16)
                        nc.scalar.copy(s16[:, :UPH], s_t[:, :UPH])
                        nc.vector.tensor_scalar(
                            out=s16[:, UPH:], in0=s_t[:, UPH:],
                            scalar1=1.0, scalar2=0.0,
                            op0=ALU.mult, op1=ALU.add,
                        )
                        eT = et_pool.tile([126, Q_F], F16)
                        nc.scalar.activation(eT[:], t_t[:], AF.Exp)
                        # DVE order: Schraudolph eS first (zs matmuls need
                        # only this), then d, then pp. (tried
                        # scalar_tensor_tensor for d/pp: cost model says
                        # 4x_2p but HW ran it at ~1x; tried gpsimd for the
                        # tensor_scalar: 2.2x slower than DVE + SBUF
                        # contention slowed DVE's tensor_tensor 1.6x)
                        i16 = is_pool.tile([126, Q_F], I16)
                        nc.vector.tensor_scalar(
                            out=i16[:], in0=s16[:],
                            scalar1=SCH16_A, scalar2=SCH16_B,
                            op0=ALU.mult, op1=ALU.add,
                        )
                        eS = i16[:].bitcast(F16)
                        d = dd_pool.tile([126, Q_F], F16)
                        nc.vector.tensor_sub(d[:], t_t[:], s16[:])
                        pp = pp_pool.tile([126, Q_F], F16)
                        nc.vector.tensor_mul(pp[:], eT[:], d[:])
                        # matmuls grouped by field in producer-readiness
                        # order: zs (needs i8 only), then zt (eT), then wm
                        # (pp) — longer dependency-free runs on PE
                        for j in range(nsl):
                            k = q * nsl + j
                            cs = slice(j * SL, (j + 1) * SL)
                            nc.tensor.matmul(
                                zs_bank[:, :], ones_t[:, k, :],
                                eS[:, cs],
                                start=(k == 0), stop=(k == K_PER_PACK - 1),
                            )
                        for j in range(nsl):
                            k = q * nsl + j
                            cs = slice(j * SL, (j + 1) * SL)
                            nc.tensor.matmul(
                                zt_bank[:, :], ones_t[:, k, :], eT[:, cs],
                                start=(k == 0), stop=(k == K_PER_PACK - 1),
                            )
                        for j in range(nsl):
                            k = q * nsl + j
                            cs = slice(j * SL, (j + 1) * SL)
                            nc.tensor.matmul(
                                wm_bank[:, :], ones_t[:, k, :], pp[:, cs],
                                start=(k == 0), stop=(k == K_PER_PACK - 1),
                            )
                    # PSUM drain on ACT (keeps DVE free to pace pp),
                    # scaled 1/16 into fp8
                    cop = cop_pool.tile([PACK_ROWS, NQ, SL], F8)
                    nc.scalar.activation(cop[:, 0, :], zt_bank[:], AF.Copy,
                                         scale=0.0625)
                    nc.scalar.activation(cop[:, 1, :], wm_bank[:], AF.Copy,
                                         scale=0.0625)
                    nc.scalar.activation(cop[:, 2, :], zs_bank[:], AF.Copy,
                                         scale=0.0625)
                    nc.sync.dma_start(out=out_ap[b, p], in_=cop[:])

    _dedupe_ldweights(nc)
    nc.compile()
    return nc


def _dedupe_ldweights(nc):
    """Remove back-to-back InstLdweights that reload the weights already in
    the PE array (zt/wm matmul pairs share the same ones lhsT). Any sem
    waits/updates on a removed load are merged into the next Matmult; the
    compile passes that run afterwards handle >1-wait splitting."""
    removed = 0
    for fn in nc.m.functions:
        for blk in fn.blocks:
            insts = list(blk.instructions)
            keep = []
            loaded = None
            pending = []
            for inst in insts:
                if isinstance(inst, mybir.InstLdweights):
                    sig = (
                        str(inst.ins[0]),
                        str(getattr(inst, "perf_mode", None)),
                        str(getattr(inst, "tile_position", None)),
                    )
                    if sig == loaded:
                        si = inst.sync_info
                        if si is not None and (
                            len(si.on_wait) > 0 or len(si.on_update) > 0
                        ):
                            pending.append(si)
                        removed += 1
                        continue
                    loaded = sig
                    keep.append(inst)
                    continue
                if isinstance(inst, mybir.InstMatmult) and pending:
                    si = inst.sync_info
                    if si is None:
                        si = mybir.SyncInfo(on_wait=[], on_update=[])
                        inst.sync_info = si
                    for p in pending:
                        si.on_wait = list(si.on_wait) + list(p.on_wait)
                        si.on_update = list(si.on_update) + list(p.on_update)
                    pending = []
                keep.append(inst)
            if len(keep) != len(insts):
                blk.instructions[:] = keep
    return removed


def _get_nc():
    if "nc" not in _NC_CACHE:
        _NC_CACHE["nc"] = _build_nc()
    return _NC_CACHE["nc"]


def _ones_blk():
    o = np.zeros((126, K_PER_PACK, PACK_ROWS), dtype=np.float16)
    r = np.arange(126)
    for k in range(K_PER_PACK):
        o[r, k, G * k + r // C] = 1.0
    return o


def kernel(preds_S, preds_T, gt_labels, _results_hook=None):
    import ml_dtypes

    S = np.maximum(
        np.asarray(preds_S, dtype=np.float32), np.float32(SCH_CLAMP)
    ).astype(ml_dtypes.float8_e4m3fn).reshape(B, C, N_TOT)
    T = np.asarray(preds_T, dtype=np.float16).reshape(B, C, N_TOT)
    labels = np.asarray(gt_labels).reshape(B, N_TOT)

    nc = _get_nc()
    ones = _ones_blk()
    in_maps = []
    for m in range(NCORES):
        sl = slice(m * NC_VOX, (m + 1) * NC_VOX)
        in_maps.append(
            {
                "s": np.ascontiguousarray(S[:, :, sl]),
                "t": np.ascontiguousarray(T[:, :, sl]),
                "ones_blk": ones,
            }
        )

    res = run_bass_kernel_spmd(nc, in_maps, list(range(NCORES)))
    if _results_hook is not None:
        _results_hook(res)

    # reassemble ZT/W/ZS into [B, N_TOT] voxel order:
    # out[b, p, 9k+g, f, v] <-> voxel (core m) m*NC_VOX + g*GL + p*PACK_F + k*SL + v
    fields = np.empty((NQ, B, N_TOT), dtype=np.float32)
    for m in range(NCORES):
        zws = res.results[m]["zws"].astype(np.float32) * 16.0
        a = zws.reshape(B, N_PACKS, K_PER_PACK, G, NQ, SL)
        # -> [NQ, B, G, N_PACKS, K_PER_PACK, SL] -> [NQ, B, NC_VOX]
        a = a.transpose(4, 0, 3, 1, 2, 5).reshape(NQ, B, NC_VOX)
        fields[:, :, m * NC_VOX : (m + 1) * NC_VOX] = a

    ZT, W, ZS = fields[0], fields[1], fields[2]
    kl = W / ZT + np.log(ZS) - np.log(ZT)

    # host finale: segment sums per (batch, class), masked mean, class 0 excluded
    loss = 0.0
    for b in range(B):
        lab = labels[b].astype(np.int64)
        sums = np.bincount(lab, weights=kl[b].astype(np.float64), minlength=C)
        counts = np.bincount(lab, minlength=C)
        terms = np.where(counts > 0, sums / (C * np.maximum(counts, 1)), 0.0)
        loss += terms[1:].sum()
    return np.float32(loss)
